# revision 27
# baseline (speedup 1.0000x reference)
"""GroupedQueryAttention TRN2 Bass kernel (v4).

Strategy (8 NeuronCores, tensor-parallel over heads):
  - Each core owns 4 q-heads (one kv head, GQA group of 4).
  - Host pre-transposes x and the weight slices into the exact on-chip
    layouts (chunk-contiguous, bf16) so every DMA is line-rate.
  - Pipeline per batch: QKV projection chunks (xT prefetched one chunk
    ahead) -> whole-batch in-place bf16 RoPE on DVE -> attention per
    (q-block, head-pair):
      paired score matmuls on PE row groups 0-63/64-127 (concurrent),
      one exp per k-tile covering both heads (strided AP on diagonal
      tiles restricts to valid columns), a 128x128 triangle mask mul,
      V carries a ones column so the softmax denominator falls out of
      the same AV matmul. The k-loop is software-pipelined (scores of
      tile j+1 issue before AV of tile j) so the PE never waits on exp.
  - 1/denominator via exp(-ln(x)) on ScalarE; the activation-table map
    is patched so exp and ln share one resident table set.
  - AllGather of ctx^T in 8x 512-token chunks; Wo-projection chunks are
    interleaved into attention-b1 PE gaps; only the last chunk's
    collective is exposed at the tail.
"""

import os
import sys

import numpy as np


def _ensure_concourse():
    try:
        import concourse.bass  # noqa: F401
    except ImportError:
        for p in ("/opt/trn_rl_repo", "/root/.axon_site/_ro/trn_rl_repo"):
            if os.path.isdir(p) and p not in sys.path:
                sys.path.insert(0, p)
        import concourse.bass  # noqa: F401


FULL_CFG = dict(B=2, S=2048, E=2048, NH=32, NKV=8, HD=64, ncores=8,
                IC=512, IC2=512, QB=512, KB=128,
                lnexp=True, diag=True, dbg=False)

LAST_RESULTS = None  # BassKernelResults of the most recent kernel() call


def _patch_act_tables():
    """Make Exp and Ln resolve to the single joint table set.

    The act-table-load pass picks, per activation, some set containing its
    function; with Exp and Ln in different sets it reloads tables (~2.7us)
    on every switch. Removing exp/ln from every set except the joint
    `natural_log_exp_and_others` (order and size preserved, so
    act_func_set_ids still match act_info.json) forces one resident set.
    """
    from concourse import bacc, hw_specs
    import concourse.mybir as mybir
    if getattr(hw_specs, "_gqa_joint_act", False):
        return
    orig = hw_specs.get_activation_tables
    Exp = mybir.ActivationFunctionType.Exp
    Ln = mybir.ActivationFunctionType.Ln

    def patched(arch):
        tabs = orig(arch)
        joint = [n for n, fns in tabs.items() if Exp in fns and Ln in fns]
        if joint:
            for n, fns in tabs.items():
                if n != joint[0]:
                    fns.discard(Exp)
                    fns.discard(Ln)
        return tabs

    hw_specs.get_activation_tables = patched
    for mod in (bacc,):
        if getattr(mod, "get_activation_tables", None) is orig:
            mod.get_activation_tables = patched
    hw_specs._gqa_joint_act = True


def build_gqa(cfg):
    """Build the Bass module for one core's SPMD program. Returns nc."""
    _ensure_concourse()
    from contextlib import ExitStack

    import concourse.mybir as mybir
    import concourse.tile as tile
    from concourse import bacc

    _patch_act_tables()

    dt = mybir.dt
    f32 = dt.float32
    bf16 = dt.bfloat16
    Exp = mybir.ActivationFunctionType.Exp
    Ln = mybir.ActivationFunctionType.Ln

    B, S, E = cfg["B"], cfg["S"], cfg["E"]
    NH, NKV, HD = cfg["NH"], cfg["NKV"], cfg["HD"]
    NCORES = cfg["ncores"]
    HPC = NH // NCORES          # q heads per core
    assert HPC == 4 and HD == 64
    QH = HPC * HD               # 256: per-core q/ctx/out rows
    KVD = 2 * HD                # 128: packed K|V projection width
    NI = B * S                  # total tokens
    ET = E // 128               # contraction tiles
    IC = cfg["IC"]              # proj token chunk (512)
    QB = cfg["QB"]              # attention q block (512)
    KB = cfg["KB"]              # attention k block (128)
    NQT = S // QB               # q blocks per batch (4)
    SKT = S // KB               # k tiles per batch (16)
    NCH = S // IC               # proj chunks per batch (4)
    NCHB = NI // IC             # proj chunks total (8)
    AGCH = 512                  # tokens per AllGather chunk
    NAG = NI // AGCH            # 8 AllGather chunks
    scale = 1.0 / float(np.sqrt(HD))
    diag = cfg["diag"]
    lnexp = cfg["lnexp"]

    nc = bacc.Bacc("TRN2", target_bir_lowering=False, debug=False,
                   num_devices=NCORES)

    xTc = nc.dram_tensor("xTc", [NCHB, 128, ET, IC], bf16,
                         kind="ExternalInput").ap()
    wqT = nc.dram_tensor("wqT", [128, ET, QH], bf16,
                         kind="ExternalInput").ap()
    wkvT = nc.dram_tensor("wkvT", [128, ET, KVD], bf16,
                          kind="ExternalInput").ap()
    woT = nc.dram_tensor("woT", [128, ET, QH], bf16,
                         kind="ExternalInput").ap()
    cosT = nc.dram_tensor("cosT", [128, S], bf16, kind="ExternalInput").ap()
    sinT = nc.dram_tensor("sinT", [128, S], bf16, kind="ExternalInput").ap()
    outT = nc.dram_tensor("outT", [QH, NI], f32, kind="ExternalOutput").ap()
    dbg = cfg.get("dbg", False)
    if dbg:
        qtd = nc.dram_tensor("qtd", [2, 128, NI], bf16,
                             kind="ExternalOutput").ap()
        ktd = nc.dram_tensor("ktd", [128, NI], bf16,
                             kind="ExternalOutput").ap()
        vaugd = nc.dram_tensor("vaugd", [128, B * SKT, HD + 1], bf16,
                               kind="ExternalOutput").ap()
        cxd = nc.dram_tensor("cxd", [2, 128, NI], bf16,
                             kind="ExternalOutput").ap()

    with tile.TileContext(nc) as tc, ExitStack() as persist:
        const = persist.enter_context(tc.tile_pool(name="const", bufs=1))
        qt_pool = persist.enter_context(tc.tile_pool(name="qt", bufs=1))
        kt_pool = persist.enter_context(tc.tile_pool(name="kt", bufs=1))
        vaug_pool = persist.enter_context(tc.tile_pool(name="vaug", bufs=1))
        ctxsb_pool = persist.enter_context(tc.tile_pool(name="ctxsb", bufs=1))
        scores_ps = persist.enter_context(
            tc.tile_pool(name="scores_ps", bufs=2, space="PSUM"))
        ctx_ps_pool = persist.enter_context(
            tc.tile_pool(name="ctx_ps", bufs=2, space="PSUM"))
        dram = persist.enter_context(
            tc.tile_pool(name="dram", bufs=1, space="DRAM"))
        e_pool = persist.enter_context(tc.tile_pool(name="e", bufs=4))
        den_pool = persist.enter_context(tc.tile_pool(name="den", bufs=2))
        rbc_pool = persist.enter_context(tc.tile_pool(name="rbc", bufs=4))

        # ---- phase-limited pools
        ph_proj = ExitStack()
        xt_pool = ph_proj.enter_context(tc.tile_pool(name="xt", bufs=2))
        proj_ps = ph_proj.enter_context(
            tc.tile_pool(name="proj_ps", bufs=2, space="PSUM"))
        vs_pool = ph_proj.enter_context(tc.tile_pool(name="vs", bufs=2))
        rope_pool = ph_proj.enter_context(tc.tile_pool(name="tmp", bufs=2))

        # ---- constants needed by the first proj chunk (DMA priority order)
        wq_sb = const.tile([128, ET, QH], bf16, name="wq_sb", tag="wq")
        wkv_sb = const.tile([128, ET, KVD], bf16, name="wkv_sb", tag="wkv")
        nc.sync.dma_start(wkv_sb[:, :, :], wkvT)
        cos_sb = const.tile([128, S], bf16, name="cos_sb", tag="cos")
        sin_sb = const.tile([128, S], bf16, name="sin_sb", tag="sin")
        wo_sb = const.tile([128, ET, QH], bf16, name="wo_sb", tag="wo")
        tri = const.tile([128, 128], bf16, name="tri", tag="tri")
        if not diag:
            nqb = QB // KB
            mask4 = const.tile([128, nqb, QB], bf16, name="mask4", tag="mask4")
        from concourse.masks import make_identity
        ident = const.tile([64, 64], bf16, name="ident", tag="ident")

        def emit_late_consts():
            """Constants not needed by the first proj chunk: emitted after its
            xT DMA so that isn't queued behind them."""
            nc.sync.dma_start(cos_sb[:, :], cosT)
            nc.sync.dma_start(sin_sb[:, :], sinT)
            nc.sync.dma_start(wo_sb[:, :, :], woT)
            make_identity(nc, ident[:, :])
            # triangle mask [128,128]: keep where q >= k
            nc.gpsimd.memset(tri[:, :], 1.0)
            nc.gpsimd.affine_select(
                out=tri[:, :], in_=tri[:, :],
                pattern=[[1, 128]], compare_op=mybir.AluOpType.is_ge,
                fill=0.0, base=0, channel_multiplier=-1)
            if not diag:
                nc.gpsimd.memset(mask4[:, :, :], 1.0)
                for j in range(nqb):
                    nc.gpsimd.affine_select(
                        out=mask4[:, j, :], in_=mask4[:, j, :],
                        pattern=[[1, QB]], compare_op=mybir.AluOpType.is_ge,
                        fill=0.0, base=-KB * j, channel_multiplier=-1)

        # ---- persistent activations
        qt_sb = [qt_pool.tile([128, NI], bf16, name=f"qt{m}", tag=f"qt{m}")
                 for m in range(HPC // 2)]
        kt_sb = kt_pool.tile([128, NI], bf16, tag="ktd")  # K^T duplicated 2x
        # inner stride 80: xbar-transpose dst column offsets must be
        # 16-aligned (65 is not); ones column lives at 64
        vaug = vaug_pool.tile([128, B * SKT, 80], bf16, tag="vaug")
        nc.gpsimd.memset(vaug[:, :, HD:HD + 1], 1.0)
        ctx_sb = [ctxsb_pool.tile([128, NI], bf16, name=f"cx{m}", tag=f"cx{m}")
                  for m in range(HPC // 2)]

        # ---- collective buffers (per AG chunk)
        cc_in = [dram.tile([QH, AGCH], bf16, name=f"cci{c}", tag=f"cci{c}")
                 for c in range(NAG)]
        cc_out = [dram.tile([E, AGCH], bf16, name=f"cco{c}", tag=f"cco{c}",
                            addr_space="Shared")
                  for c in range(NAG)]

        # ================= building blocks =================

        def load_xt(b, ci):
            xt = xt_pool.tile([128, ET, IC], bf16, name="xt", tag="xt")
            nc.sync.dma_start(xt[:, :, :], xTc[b * NCH + ci, :, :, :])
            return xt

        def proj_kv_group(b, ci, xt):
            i0 = b * S + ci * IC
            kv_ps = proj_ps.tile([128, IC], f32, name="pps", tag="proj")
            for t in range(ET):
                nc.tensor.matmul(kv_ps[:, :], wkv_sb[:, t, :], xt[:, t, :],
                                 start=(t == 0), stop=(t == ET - 1))
            nc.vector.tensor_copy(kt_sb[0:64, i0:i0 + IC], kv_ps[0:64, :])
            vs = vs_pool.tile([64, IC], bf16, name="vs", tag="vs")
            nc.vector.tensor_copy(vs[:, :], kv_ps[64:128, :])
            # V transpose on the PE (bf16 PSUM out), one DVE copy into vaug.
            # (DMA xbar transposes serialize the whole Sync ring against
            # in-flight collectives - keep them off the DMA rings entirely.)
            kidx0 = (b * S + ci * IC) // KB
            vt = scores_ps.tile([128, IC // KB, 64], bf16, name="vt", tag="s")
            for j in range(IC // KB):
                nc.tensor.transpose(vt[:, j, :], vs[:, j * KB:(j + 1) * KB],
                                    ident[:, :])
            nc.vector.tensor_copy(vaug[:, kidx0:kidx0 + IC // KB, 0:HD],
                                  vt[:, :, :])

        def proj_q_group(b, ci, m, xt):
            i0 = b * S + ci * IC
            q_ps = proj_ps.tile([128, IC], f32, name="pps", tag="proj")
            for t in range(ET):
                nc.tensor.matmul(
                    q_ps[:, :], wq_sb[:, t, m * 128:(m + 1) * 128],
                    xt[:, t, :], start=(t == 0), stop=(t == ET - 1))
            nc.vector.tensor_copy(qt_sb[m][:, i0:i0 + IC], q_ps[:, :])

        def proj_compute(b, ci, xt):
            """QKV projection for tokens [b*S + ci*IC, +IC). kv first."""
            proj_kv_group(b, ci, xt)
            proj_q_group(b, ci, 0, xt)
            proj_q_group(b, ci, 1, xt)

        def rope_inplace(dst, parts, b):
            """dst[0:parts, b*S:(b+1)*S] = dst*cos + swap32(dst)*signed_sin."""
            sl = slice(b * S, (b + 1) * S)
            tmp = rope_pool.tile([128, S], bf16, name="tmp", tag="tmp")
            for h0 in range(0, parts, 64):
                nc.vector.tensor_copy(tmp[h0:h0 + 32, :],
                                      dst[h0 + 32:h0 + 64, sl])
                nc.vector.tensor_copy(tmp[h0 + 32:h0 + 64, :],
                                      dst[h0:h0 + 32, sl])
            nc.vector.tensor_mul(tmp[0:parts, :], tmp[0:parts, :],
                                 sin_sb[0:parts, :])
            nc.vector.tensor_mul(dst[0:parts, sl], dst[0:parts, sl],
                                 cos_sb[0:parts, :])
            nc.vector.tensor_add(dst[0:parts, sl], dst[0:parts, sl],
                                 tmp[0:parts, :])

        def rope_k(b):
            rope_inplace(kt_sb, 64, b)
            nc.vector.tensor_copy(kt_sb[64:128, b * S:(b + 1) * S],
                                  kt_sb[0:64, b * S:(b + 1) * S])

        def attn_qt(b, qt, fillers=(), budget=(0, 0)):
            """Attention for q-block qt of batch b, both head pairs.

            The k-loop is software-pipelined: scores/exp for tile kt issue
            before the AV matmuls of tile kt-1, so the in-order PE queue
            never blocks on the exp of the tile it just scored.

            `fillers` is a deque of callables emitting independent PE work
            (proj / Wo-projection groups); `budget` = (units at the m0->m1
            boundary, units after m1) absorbs the ACT-over-PE surplus of
            this window inside the in-order PE queue.
            """
            qoff = b * S + qt * QB
            nkt = (qt + 1) * (QB // KB)
            for m in range(HPC // 2):
                if m == 1:
                    for _ in range(budget[0]):
                        if fillers:
                            fillers.popleft()()
                ctxA = ctx_ps_pool.tile([128, QB], f32, name="ctx", tag="ctx")
                ctxB = ctx_ps_pool.tile([128, QB], f32, name="ctx", tag="ctx")

                def av_pair(e_t, c0, kt):
                    nc.tensor.matmul(ctxA[0:HD + 1, c0:QB],
                                     vaug[:, b * SKT + kt, 0:HD + 1],
                                     e_t[:, c0:QB],
                                     start=(kt == 0), stop=(kt == nkt - 1))
                    nc.tensor.matmul(ctxB[0:HD + 1, c0:QB],
                                     vaug[:, b * SKT + kt, 0:HD + 1],
                                     e_t[:, QB + c0:2 * QB],
                                     start=(kt == 0), stop=(kt == nkt - 1))

                pend = None
                for kt in range(nkt):
                    koff = b * S + kt * KB
                    j = kt - qt * (QB // KB)
                    c0 = 128 * j if (diag and j >= 0) else 0
                    s_ps = scores_ps.tile([128, 2 * QB], f32,
                                          name="s_ps", tag="s")
                    nc.tensor.matmul(s_ps[:, c0:QB],
                                     kt_sb[0:64, koff:koff + KB],
                                     qt_sb[m][0:64, qoff + c0:qoff + QB],
                                     start=True, stop=True)
                    nc.tensor.matmul(s_ps[:, QB + c0:2 * QB],
                                     kt_sb[64:128, koff:koff + KB],
                                     qt_sb[m][64:128, qoff + c0:qoff + QB],
                                     start=True, stop=True)
                    e_t = e_pool.tile([128, 2 * QB], bf16, name="e_t", tag="e")
                    if c0 == 0:
                        nc.scalar.activation(e_t[:, :], s_ps[:, :], Exp,
                                             scale=scale)
                    else:
                        # one strided-AP call over both heads' valid columns
                        sap = s_ps[:, :].rearrange(
                            "p (h q) -> p h q", h=2)[:, :, c0:QB]
                        eap = e_t[:, :].rearrange(
                            "p (h q) -> p h q", h=2)[:, :, c0:QB]
                        nc.scalar.activation(eap, sap, Exp, scale=scale)
                    if j >= 0:  # diagonal tile: triangle mask
                        if diag:
                            nc.vector.tensor_mul(e_t[:, c0:c0 + KB],
                                                 e_t[:, c0:c0 + KB],
                                                 tri[:, :])
                            nc.vector.tensor_mul(e_t[:, QB + c0:QB + c0 + KB],
                                                 e_t[:, QB + c0:QB + c0 + KB],
                                                 tri[:, :])
                        else:
                            nc.vector.tensor_mul(e_t[:, 0:QB], e_t[:, 0:QB],
                                                 mask4[:, j, :])
                            nc.vector.tensor_mul(e_t[:, QB:2 * QB],
                                                 e_t[:, QB:2 * QB],
                                                 mask4[:, j, :])
                    if pend is not None:
                        av_pair(*pend)
                    pend = (e_t, c0, kt)
                av_pair(*pend)

                # softmax denominators -> 1/x -> normalize + cast to bf16.
                # Both denominators packed along the free axis on partition 0
                # (cross-partition-base engine ops misbehave).
                den = den_pool.tile([1, 2 * QB], f32, name="den", tag="den")
                nc.vector.tensor_copy(den[0:1, 0:QB], ctxA[HD:HD + 1, :])
                nc.vector.tensor_copy(den[0:1, QB:2 * QB], ctxB[HD:HD + 1, :])
                rec = den_pool.tile([1, 2 * QB], f32, name="rec", tag="rec")
                if lnexp:
                    ld = den_pool.tile([1, 2 * QB], f32, name="ld", tag="ld")
                    nc.scalar.activation(ld[0:1, :], den[0:1, :], Ln)
                    nc.scalar.activation(rec[0:1, :], ld[0:1, :], Exp,
                                         scale=-1.0)
                else:
                    nc.vector.reciprocal(rec[0:1, :], den[0:1, :])
                rbcA = rbc_pool.tile([64, QB], f32, name="rbc", tag="rbc")
                nc.gpsimd.partition_broadcast(rbcA[:, :], rec[0:1, 0:QB])
                rbcB = rbc_pool.tile([64, QB], f32, name="rbc", tag="rbc")
                nc.gpsimd.partition_broadcast(rbcB[:, :], rec[0:1, QB:2 * QB])
                nc.vector.tensor_mul(ctx_sb[m][0:64, qoff:qoff + QB],
                                     ctxA[0:HD, :], rbcA[:, :])
                nc.vector.tensor_mul(ctx_sb[m][64:128, qoff:qoff + QB],
                                     ctxB[0:HD, :], rbcB[:, :])
                # stage into the AllGather input buffer (gpsimd ring keeps
                # this dependent DMA off the Sync queue's in-order stream)
                c = (b * S + qt * QB) // AGCH
                nc.gpsimd.dma_start(cc_in[c][m * 128:(m + 1) * 128, :],
                                    ctx_sb[m][:, qoff:qoff + QB])
            for _ in range(budget[1]):
                if fillers:
                    fillers.popleft()()

        def ag_chunk(c):
            nc.gpsimd.collective_compute(
                "AllGather", mybir.AluOpType.bypass,
                replica_groups=[list(range(NCORES))],
                ins=[cc_in[c][:, :]],
                outs=[cc_out[c][:, :]])

        # ================= emission schedule =================
        from collections import deque

        x0 = load_xt(0, 0)
        nc.sync.dma_start(wq_sb[:, :, :], wqT)
        emit_late_consts()
        x1 = load_xt(0, 1)
        proj_compute(0, 0, x0)
        x2 = load_xt(0, 2)
        proj_compute(0, 1, x1)
        x3 = load_xt(0, 3)
        proj_compute(0, 2, x2)
        proj_compute(0, 3, x3)
        rope_k(0)
        rope_inplace(qt_sb[0], 128, 0)
        rope_inplace(qt_sb[1], 128, 0)

        # proj(b1) as filler units popped inside attention-b0 windows
        xts1 = {0: load_xt(1, 0)}

        def kv_unit(ci):
            def f():
                if ci + 1 < NCH and ci + 1 not in xts1:
                    xts1[ci + 1] = load_xt(1, ci + 1)
                proj_kv_group(1, ci, xts1[ci])
            return f

        def q_unit(ci, m):
            def f():
                proj_q_group(1, ci, m, xts1[ci])
            return f

        units = deque()
        for ci in range(NCH):
            units.append(kv_unit(ci))
            units.append(q_unit(ci, 0))
            units.append(q_unit(ci, 1))

        attn_qt(0, 0, units, (1, 1))
        ag_chunk(0)
        attn_qt(0, 1, units, (1, 2))
        ag_chunk(1)
        attn_qt(0, 2, units, (2, 2))
        ag_chunk(2)
        attn_qt(0, 3, units, (2, 1))
        ag_chunk(3)
        while units:
            units.popleft()()
        rope_k(1)
        rope_inplace(qt_sb[0], 128, 1)
        rope_inplace(qt_sb[1], 128, 1)
        ph_proj.close()

        # phase 5 pools (PSUM banks reuse the closed proj pool's)
        with ExitStack() as ph5:
            o_ps_pool = ph5.enter_context(
                tc.tile_pool(name="o_ps", bufs=2, space="PSUM"))
            ct_pool = ph5.enter_context(tc.tile_pool(name="ct", bufs=2))
            ob_pool = ph5.enter_context(tc.tile_pool(name="ob", bufs=2))
            cts = {}

            def ph5_mgroup(c, m, ct):
                tok0 = c * AGCH
                o_ps = o_ps_pool.tile([128, AGCH], f32, name="ops", tag="ops")
                for t in range(ET):
                    nc.tensor.matmul(
                        o_ps[:, :], wo_sb[:, t, m * 128:(m + 1) * 128],
                        ct[:, t, :], start=(t == 0), stop=(t == ET - 1))
                ob = ob_pool.tile([128, AGCH], f32, name="ob", tag="ob")
                nc.vector.tensor_copy(ob[:, :], o_ps[:, :])
                nc.sync.dma_start(
                    outT[m * 128:(m + 1) * 128, tok0:tok0 + AGCH], ob[:, :])

            def ct_m0_unit(c):
                def f():
                    ct = ct_pool.tile([128, ET, AGCH], bf16, name="ct",
                                      tag="ct")
                    nc.sync.dma_start(
                        ct[:, :, :],
                        cc_out[c][:, :].rearrange("(t p) i -> p t i", p=128))
                    cts[c] = ct
                    ph5_mgroup(c, 0, ct)
                return f

            def m1_unit(c):
                def f():
                    ph5_mgroup(c, 1, cts[c])
                return f

            u5 = deque()
            for c in (0, 1, 2, 3, 4, 5, 7, 6):
                u5.append(ct_m0_unit(c))
                u5.append(m1_unit(c))

            # fill the rope-b1 PE window with the first three Wo chunks
            for _ in range(6):
                u5.popleft()()
            # attention b1; Wo chunks popped into each window's ACT surplus.
            # A chunk's units must never be emitted before its ag_chunk: a
            # reader emitted before its writer gets no RAW dependency and
            # reads uninitialized DRAM.
            attn_qt(1, 0, u5, (1, 1))
            ag_chunk(4)
            attn_qt(1, 1, u5, (1, 1))
            ag_chunk(5)
            # qt3 before qt2: the exposed tail chain then hangs off the
            # smaller qt2 window (12 k-tiles of ACT backlog, not 16)
            attn_qt(1, 3, u5, (2, 0))
            ag_chunk(7)
            attn_qt(1, 2, u5, (2, 0))
            ag_chunk(6)
            while u5:
                u5.popleft()()

            if dbg:
                for m in range(2):
                    nc.sync.dma_start(qtd[m, :, :], qt_sb[m][:, :])
                    nc.sync.dma_start(cxd[m, :, :], ctx_sb[m][:, :])
                nc.sync.dma_start(ktd[:, :], kt_sb[:, :])
                nc.sync.dma_start(vaugd[:, :, :], vaug[:, :, 0:HD + 1])

    nc.compile()
    return nc


def make_in_maps(cfg, x, cos, sin, Wq, Wk, Wv, Wo):
    """Host-side prep: transpose/slice full inputs into per-core input maps.

    All matmul operands are pre-rearranged into the on-chip [128, t, free]
    layouts so every device DMA is contiguous per partition.
    """
    import ml_dtypes
    B, S, E = cfg["B"], cfg["S"], cfg["E"]
    NH, NKV, HD, NCORES = cfg["NH"], cfg["NKV"], cfg["HD"], cfg["ncores"]
    IC = cfg["IC"]
    HPC = NH // NCORES
    QH = HPC * HD
    KVPC = NKV // NCORES
    ET = E // 128
    NCHB = (B * S) // IC

    mmnp = ml_dtypes.bfloat16
    x = np.asarray(x, dtype=np.float32)
    cos = np.asarray(cos, dtype=np.float32)
    sin = np.asarray(sin, dtype=np.float32)
    Wq = np.asarray(Wq, dtype=np.float32)
    Wk = np.asarray(Wk, dtype=np.float32)
    Wv = np.asarray(Wv, dtype=np.float32)
    Wo = np.asarray(Wo, dtype=np.float32)

    xT = x.reshape(B * S, E).T.astype(mmnp)              # [E, NI]
    xTc = np.ascontiguousarray(
        xT.reshape(ET, 128, NCHB, IC).transpose(2, 1, 0, 3))

    def wprep(w):  # [E, out] -> [128, ET, out] contiguous
        return np.ascontiguousarray(
            w.reshape(ET, 128, -1).transpose(1, 0, 2).astype(mmnp))

    cos_t = cos.T[:HD]                        # [64, S]
    cosT = np.ascontiguousarray(
        np.concatenate([cos_t, cos_t], axis=0).astype(mmnp))
    sin_t = sin.T[:HD].copy()
    sin_t[:HD // 2] *= -1.0                   # signed sin for rotate-half
    sinT = np.ascontiguousarray(
        np.concatenate([sin_t, sin_t], axis=0).astype(mmnp))

    in_maps = []
    for c in range(NCORES):
        qsl = slice(c * QH, (c + 1) * QH)
        ksl = slice(c * KVPC * HD, (c + 1) * KVPC * HD)
        wq = wprep(Wq[qsl, :].T)
        wkv = wprep(np.concatenate([Wk[ksl, :].T, Wv[ksl, :].T], axis=1))
        wo = wprep(Wo[qsl, :].T)
        in_maps.append(dict(xTc=xTc, wqT=wq, wkvT=wkv, woT=wo,
                            cosT=cosT, sinT=sinT))
    return in_maps


def assemble_output(cfg, results):
    B, S, E = cfg["B"], cfg["S"], cfg["E"]
    outT = np.concatenate([r["outT"] for r in results], axis=0)  # [E, B*S]
    return np.ascontiguousarray(outT.T.reshape(B, S, E).astype(np.float32))


def kernel(x, mask, cos, sin, Wq, Wk, Wv, Wo):
    global LAST_RESULTS
    _ensure_concourse()
    from concourse import bass_utils

    cfg = FULL_CFG
    nc = build_gqa(cfg)
    in_maps = make_in_maps(cfg, x, cos, sin, Wq, Wk, Wv, Wo)
    res = bass_utils.run_bass_kernel_spmd(
        nc, in_maps, core_ids=list(range(cfg["ncores"])))
    LAST_RESULTS = res
    return assemble_output(cfg, res.results)


# revision 29
# speedup vs baseline: 1.0503x; 1.0503x over previous
"""GroupedQueryAttention TRN2 Bass kernel (v4).

Strategy (8 NeuronCores, tensor-parallel over heads):
  - Each core owns 4 q-heads (one kv head, GQA group of 4).
  - Host pre-transposes x and the weight slices into the exact on-chip
    layouts (chunk-contiguous, bf16) so every DMA is line-rate.
  - Pipeline per batch: QKV projection chunks (xT prefetched one chunk
    ahead) -> whole-batch in-place bf16 RoPE on DVE -> attention per
    (q-block, head-pair):
      paired score matmuls on PE row groups 0-63/64-127 (concurrent),
      one exp per k-tile covering both heads (strided AP on diagonal
      tiles restricts to valid columns), a 128x128 triangle mask mul,
      V carries a ones column so the softmax denominator falls out of
      the same AV matmul. The k-loop is software-pipelined (scores of
      tile j+1 issue before AV of tile j) so the PE never waits on exp.
  - 1/denominator via exp(-ln(x)) on ScalarE; the activation-table map
    is patched so exp and ln share one resident table set.
  - AllGather of ctx^T in 8x 512-token chunks; Wo-projection chunks are
    interleaved into attention-b1 PE gaps; only the last chunk's
    collective is exposed at the tail.
"""

import os
import sys

import numpy as np


def _ensure_concourse():
    try:
        import concourse.bass  # noqa: F401
    except ImportError:
        for p in ("/opt/trn_rl_repo", "/root/.axon_site/_ro/trn_rl_repo"):
            if os.path.isdir(p) and p not in sys.path:
                sys.path.insert(0, p)
        import concourse.bass  # noqa: F401


FULL_CFG = dict(B=2, S=2048, E=2048, NH=32, NKV=8, HD=64, ncores=8,
                IC=512, IC2=512, QB=512, KB=128,
                lnexp=True, diag=True, dbg=False)

LAST_RESULTS = None  # BassKernelResults of the most recent kernel() call


def _patch_act_tables():
    """Make Exp and Ln resolve to the single joint table set.

    The act-table-load pass picks, per activation, some set containing its
    function; with Exp and Ln in different sets it reloads tables (~2.7us)
    on every switch. Removing exp/ln from every set except the joint
    `natural_log_exp_and_others` (order and size preserved, so
    act_func_set_ids still match act_info.json) forces one resident set.
    """
    from concourse import bacc, hw_specs
    import concourse.mybir as mybir
    if getattr(hw_specs, "_gqa_joint_act", False):
        return
    orig = hw_specs.get_activation_tables
    Exp = mybir.ActivationFunctionType.Exp
    Ln = mybir.ActivationFunctionType.Ln

    def patched(arch):
        tabs = orig(arch)
        joint = [n for n, fns in tabs.items() if Exp in fns and Ln in fns]
        if joint:
            for n, fns in tabs.items():
                if n != joint[0]:
                    fns.discard(Exp)
                    fns.discard(Ln)
        return tabs

    hw_specs.get_activation_tables = patched
    for mod in (bacc,):
        if getattr(mod, "get_activation_tables", None) is orig:
            mod.get_activation_tables = patched
    hw_specs._gqa_joint_act = True


def build_gqa(cfg):
    """Build the Bass module for one core's SPMD program. Returns nc."""
    _ensure_concourse()
    from contextlib import ExitStack

    import concourse.mybir as mybir
    import concourse.tile as tile
    from concourse import bacc

    _patch_act_tables()

    dt = mybir.dt
    f32 = dt.float32
    bf16 = dt.bfloat16
    Exp = mybir.ActivationFunctionType.Exp
    Ln = mybir.ActivationFunctionType.Ln

    B, S, E = cfg["B"], cfg["S"], cfg["E"]
    NH, NKV, HD = cfg["NH"], cfg["NKV"], cfg["HD"]
    NCORES = cfg["ncores"]
    HPC = NH // NCORES          # q heads per core
    assert HPC == 4 and HD == 64
    QH = HPC * HD               # 256: per-core q/ctx/out rows
    KVD = 2 * HD                # 128: packed K|V projection width
    NI = B * S                  # total tokens
    ET = E // 128               # contraction tiles
    IC = cfg["IC"]              # proj token chunk (512)
    QB = cfg["QB"]              # attention q block (512)
    KB = cfg["KB"]              # attention k block (128)
    NQT = S // QB               # q blocks per batch (4)
    SKT = S // KB               # k tiles per batch (16)
    NCH = S // IC               # proj chunks per batch (4)
    NCHB = NI // IC             # proj chunks total (8)
    AGCH = 512                  # tokens per AllGather chunk
    NAG = NI // AGCH            # 8 AllGather chunks
    scale = 1.0 / float(np.sqrt(HD))
    diag = cfg["diag"]
    lnexp = cfg["lnexp"]

    nc = bacc.Bacc("TRN2", target_bir_lowering=False, debug=False,
                   num_devices=NCORES)

    xTc = nc.dram_tensor("xTc", [NCHB, 128, ET, IC], bf16,
                         kind="ExternalInput").ap()
    wqT = nc.dram_tensor("wqT", [128, ET, QH], bf16,
                         kind="ExternalInput").ap()
    wkvT = nc.dram_tensor("wkvT", [128, ET, KVD], bf16,
                          kind="ExternalInput").ap()
    woT = nc.dram_tensor("woT", [128, ET, QH], bf16,
                         kind="ExternalInput").ap()
    cosT = nc.dram_tensor("cosT", [128, S], bf16, kind="ExternalInput").ap()
    sinT = nc.dram_tensor("sinT", [128, S], bf16, kind="ExternalInput").ap()
    outT = nc.dram_tensor("outT", [QH, NI], f32, kind="ExternalOutput").ap()
    dbg = cfg.get("dbg", False)
    if dbg:
        qtd = nc.dram_tensor("qtd", [2, 128, NI], bf16,
                             kind="ExternalOutput").ap()
        ktd = nc.dram_tensor("ktd", [128, NI], bf16,
                             kind="ExternalOutput").ap()
        vaugd = nc.dram_tensor("vaugd", [128, B * SKT, HD + 1], bf16,
                               kind="ExternalOutput").ap()
        cxd = nc.dram_tensor("cxd", [2, 128, NI], bf16,
                             kind="ExternalOutput").ap()

    with tile.TileContext(nc) as tc, ExitStack() as persist:
        const = persist.enter_context(tc.tile_pool(name="const", bufs=1))
        qt_pool = persist.enter_context(tc.tile_pool(name="qt", bufs=1))
        kt_pool = persist.enter_context(tc.tile_pool(name="kt", bufs=1))
        vaug_pool = persist.enter_context(tc.tile_pool(name="vaug", bufs=1))
        ctxsb_pool = persist.enter_context(tc.tile_pool(name="ctxsb", bufs=1))
        scores_ps = persist.enter_context(
            tc.tile_pool(name="scores_ps", bufs=2, space="PSUM"))
        ctx_ps_pool = persist.enter_context(
            tc.tile_pool(name="ctx_ps", bufs=2, space="PSUM"))
        dram = persist.enter_context(
            tc.tile_pool(name="dram", bufs=1, space="DRAM"))
        e_pool = persist.enter_context(tc.tile_pool(name="e", bufs=4))
        den_pool = persist.enter_context(tc.tile_pool(name="den", bufs=2))
        rbc_pool = persist.enter_context(tc.tile_pool(name="rbc", bufs=4))

        # ---- phase-limited pools
        ph_proj = ExitStack()
        xt_pool = ph_proj.enter_context(tc.tile_pool(name="xt", bufs=2))
        proj_ps = ph_proj.enter_context(
            tc.tile_pool(name="proj_ps", bufs=2, space="PSUM"))
        vs_pool = ph_proj.enter_context(tc.tile_pool(name="vs", bufs=2))
        rope_pool = ph_proj.enter_context(tc.tile_pool(name="tmp", bufs=2))

        # ---- constants needed by the first proj chunk (DMA priority order)
        wq_sb = const.tile([128, ET, QH], bf16, name="wq_sb", tag="wq")
        wkv_sb = const.tile([128, ET, KVD], bf16, name="wkv_sb", tag="wkv")
        nc.sync.dma_start(wkv_sb[:, :, :], wkvT)
        cos_sb = const.tile([128, S], bf16, name="cos_sb", tag="cos")
        sin_sb = const.tile([128, S], bf16, name="sin_sb", tag="sin")
        wo_sb = const.tile([128, ET, QH], bf16, name="wo_sb", tag="wo")
        tri = const.tile([128, 128], bf16, name="tri", tag="tri")
        if not diag:
            nqb = QB // KB
            mask4 = const.tile([128, nqb, QB], bf16, name="mask4", tag="mask4")
        from concourse.masks import make_identity
        ident = const.tile([64, 64], bf16, name="ident", tag="ident")

        def emit_late_consts():
            """Constants not needed by the first proj chunk: emitted after its
            xT DMA so that isn't queued behind them."""
            nc.sync.dma_start(cos_sb[:, :], cosT)
            nc.sync.dma_start(sin_sb[:, :], sinT)
            nc.sync.dma_start(wo_sb[:, :, :], woT)
            make_identity(nc, ident[:, :])
            # triangle mask [128,128]: keep where q >= k
            nc.gpsimd.memset(tri[:, :], 1.0)
            nc.gpsimd.affine_select(
                out=tri[:, :], in_=tri[:, :],
                pattern=[[1, 128]], compare_op=mybir.AluOpType.is_ge,
                fill=0.0, base=0, channel_multiplier=-1)
            if not diag:
                nc.gpsimd.memset(mask4[:, :, :], 1.0)
                for j in range(nqb):
                    nc.gpsimd.affine_select(
                        out=mask4[:, j, :], in_=mask4[:, j, :],
                        pattern=[[1, QB]], compare_op=mybir.AluOpType.is_ge,
                        fill=0.0, base=-KB * j, channel_multiplier=-1)

        # ---- persistent activations
        qt_sb = [qt_pool.tile([128, NI], bf16, name=f"qt{m}", tag=f"qt{m}")
                 for m in range(HPC // 2)]
        kt_sb = kt_pool.tile([128, NI], bf16, tag="ktd")  # K^T duplicated 2x
        # inner stride 80: xbar-transpose dst column offsets must be
        # 16-aligned (65 is not); ones column lives at 64
        vaug = vaug_pool.tile([128, B * SKT, 80], bf16, tag="vaug")
        nc.gpsimd.memset(vaug[:, :, HD:HD + 1], 1.0)
        ctx_sb = [ctxsb_pool.tile([128, NI], bf16, name=f"cx{m}", tag=f"cx{m}")
                  for m in range(HPC // 2)]

        # ---- collective buffers (per AG chunk)
        cc_in = [dram.tile([QH, AGCH], bf16, name=f"cci{c}", tag=f"cci{c}")
                 for c in range(NAG)]
        cc_out = [dram.tile([E, AGCH], bf16, name=f"cco{c}", tag=f"cco{c}",
                            addr_space="Shared")
                  for c in range(NAG)]

        # ================= building blocks =================

        def load_xt(b, ci):
            xt = xt_pool.tile([128, ET, IC], bf16, name="xt", tag="xt")
            nc.sync.dma_start(xt[:, :, :], xTc[b * NCH + ci, :, :, :])
            return xt

        def proj_kv_group(b, ci, xt):
            i0 = b * S + ci * IC
            kv_ps = proj_ps.tile([128, IC], f32, name="pps", tag="proj")
            for t in range(ET):
                nc.tensor.matmul(kv_ps[:, :], wkv_sb[:, t, :], xt[:, t, :],
                                 start=(t == 0), stop=(t == ET - 1))
            nc.vector.tensor_copy(kt_sb[0:64, i0:i0 + IC], kv_ps[0:64, :])
            vs = vs_pool.tile([64, IC], bf16, name="vs", tag="vs")
            nc.vector.tensor_copy(vs[:, :], kv_ps[64:128, :])
            # V transpose on the PE (bf16 PSUM out), one DVE copy into vaug.
            # (DMA xbar transposes serialize the whole Sync ring against
            # in-flight collectives - keep them off the DMA rings entirely.)
            kidx0 = (b * S + ci * IC) // KB
            vt = scores_ps.tile([128, IC // KB, 64], bf16, name="vt", tag="s")
            for j in range(IC // KB):
                nc.tensor.transpose(vt[:, j, :], vs[:, j * KB:(j + 1) * KB],
                                    ident[:, :])
            nc.vector.tensor_copy(vaug[:, kidx0:kidx0 + IC // KB, 0:HD],
                                  vt[:, :, :])

        def proj_q_group(b, ci, m, xt):
            i0 = b * S + ci * IC
            q_ps = proj_ps.tile([128, IC], f32, name="pps", tag="proj")
            for t in range(ET):
                nc.tensor.matmul(
                    q_ps[:, :], wq_sb[:, t, m * 128:(m + 1) * 128],
                    xt[:, t, :], start=(t == 0), stop=(t == ET - 1))
            nc.vector.tensor_copy(qt_sb[m][:, i0:i0 + IC], q_ps[:, :])

        def proj_compute(b, ci, xt):
            """QKV projection for tokens [b*S + ci*IC, +IC). kv first."""
            proj_kv_group(b, ci, xt)
            proj_q_group(b, ci, 0, xt)
            proj_q_group(b, ci, 1, xt)

        def rope_inplace(dst, parts, b):
            """dst[0:parts, b*S:(b+1)*S] = dst*cos + swap32(dst)*signed_sin."""
            sl = slice(b * S, (b + 1) * S)
            tmp = rope_pool.tile([128, S], bf16, name="tmp", tag="tmp")
            for h0 in range(0, parts, 64):
                nc.vector.tensor_copy(tmp[h0:h0 + 32, :],
                                      dst[h0 + 32:h0 + 64, sl])
                nc.vector.tensor_copy(tmp[h0 + 32:h0 + 64, :],
                                      dst[h0:h0 + 32, sl])
            nc.vector.tensor_mul(tmp[0:parts, :], tmp[0:parts, :],
                                 sin_sb[0:parts, :])
            nc.vector.tensor_mul(dst[0:parts, sl], dst[0:parts, sl],
                                 cos_sb[0:parts, :])
            nc.vector.tensor_add(dst[0:parts, sl], dst[0:parts, sl],
                                 tmp[0:parts, :])

        def rope_k(b):
            rope_inplace(kt_sb, 64, b)
            nc.vector.tensor_copy(kt_sb[64:128, b * S:(b + 1) * S],
                                  kt_sb[0:64, b * S:(b + 1) * S])

        def attn_qt(b, qt, fillers=(), budget=(0, 0)):
            """Attention for q-block qt of batch b, both head pairs.

            The k-loop is software-pipelined: scores/exp for tile kt issue
            before the AV matmuls of tile kt-1, so the in-order PE queue
            never blocks on the exp of the tile it just scored.

            `fillers` is a deque of callables emitting independent PE work
            (proj / Wo-projection groups); `budget` = (units at the m0->m1
            boundary, units after m1) absorbs the ACT-over-PE surplus of
            this window inside the in-order PE queue.
            """
            qoff = b * S + qt * QB
            nkt = (qt + 1) * (QB // KB)
            for m in range(HPC // 2):
                if m == 1:
                    for _ in range(budget[0]):
                        if fillers:
                            fillers.popleft()()
                ctxA = ctx_ps_pool.tile([128, QB], f32, name="ctx", tag="ctx")
                ctxB = ctx_ps_pool.tile([128, QB], f32, name="ctx", tag="ctx")

                def av_pair(e_t, c0, kt):
                    nc.tensor.matmul(ctxA[0:HD + 1, c0:QB],
                                     vaug[:, b * SKT + kt, 0:HD + 1],
                                     e_t[:, c0:QB],
                                     start=(kt == 0), stop=(kt == nkt - 1))
                    nc.tensor.matmul(ctxB[0:HD + 1, c0:QB],
                                     vaug[:, b * SKT + kt, 0:HD + 1],
                                     e_t[:, QB + c0:2 * QB],
                                     start=(kt == 0), stop=(kt == nkt - 1))

                pend = None
                for kt in range(nkt):
                    koff = b * S + kt * KB
                    j = kt - qt * (QB // KB)
                    c0 = 128 * j if (diag and j >= 0) else 0
                    s_ps = scores_ps.tile([128, 2 * QB], f32,
                                          name="s_ps", tag="s")
                    nc.tensor.matmul(s_ps[:, c0:QB],
                                     kt_sb[0:64, koff:koff + KB],
                                     qt_sb[m][0:64, qoff + c0:qoff + QB],
                                     start=True, stop=True)
                    nc.tensor.matmul(s_ps[:, QB + c0:2 * QB],
                                     kt_sb[64:128, koff:koff + KB],
                                     qt_sb[m][64:128, qoff + c0:qoff + QB],
                                     start=True, stop=True)
                    e_t = e_pool.tile([128, 2 * QB], bf16, name="e_t", tag="e")
                    if c0 == 0:
                        nc.scalar.activation(e_t[:, :], s_ps[:, :], Exp,
                                             scale=scale)
                    else:
                        # one strided-AP call over both heads' valid columns
                        sap = s_ps[:, :].rearrange(
                            "p (h q) -> p h q", h=2)[:, :, c0:QB]
                        eap = e_t[:, :].rearrange(
                            "p (h q) -> p h q", h=2)[:, :, c0:QB]
                        nc.scalar.activation(eap, sap, Exp, scale=scale)
                    if j >= 0:  # diagonal tile: triangle mask
                        if diag:
                            nc.vector.tensor_mul(e_t[:, c0:c0 + KB],
                                                 e_t[:, c0:c0 + KB],
                                                 tri[:, :])
                            nc.vector.tensor_mul(e_t[:, QB + c0:QB + c0 + KB],
                                                 e_t[:, QB + c0:QB + c0 + KB],
                                                 tri[:, :])
                        else:
                            nc.vector.tensor_mul(e_t[:, 0:QB], e_t[:, 0:QB],
                                                 mask4[:, j, :])
                            nc.vector.tensor_mul(e_t[:, QB:2 * QB],
                                                 e_t[:, QB:2 * QB],
                                                 mask4[:, j, :])
                    if pend is not None:
                        av_pair(*pend)
                    pend = (e_t, c0, kt)
                av_pair(*pend)

                # softmax denominators -> 1/x -> normalize + cast to bf16.
                # Both denominators packed along the free axis on partition 0
                # (cross-partition-base engine ops misbehave).
                den = den_pool.tile([1, 2 * QB], f32, name="den", tag="den")
                nc.vector.tensor_copy(den[0:1, 0:QB], ctxA[HD:HD + 1, :])
                nc.vector.tensor_copy(den[0:1, QB:2 * QB], ctxB[HD:HD + 1, :])
                rec = den_pool.tile([1, 2 * QB], f32, name="rec", tag="rec")
                if lnexp:
                    ld = den_pool.tile([1, 2 * QB], f32, name="ld", tag="ld")
                    nc.scalar.activation(ld[0:1, :], den[0:1, :], Ln)
                    nc.scalar.activation(rec[0:1, :], ld[0:1, :], Exp,
                                         scale=-1.0)
                else:
                    nc.vector.reciprocal(rec[0:1, :], den[0:1, :])
                rbcA = rbc_pool.tile([64, QB], f32, name="rbc", tag="rbc")
                nc.gpsimd.partition_broadcast(rbcA[:, :], rec[0:1, 0:QB])
                rbcB = rbc_pool.tile([64, QB], f32, name="rbc", tag="rbc")
                nc.gpsimd.partition_broadcast(rbcB[:, :], rec[0:1, QB:2 * QB])
                nc.vector.tensor_mul(ctx_sb[m][0:64, qoff:qoff + QB],
                                     ctxA[0:HD, :], rbcA[:, :])
                nc.vector.tensor_mul(ctx_sb[m][64:128, qoff:qoff + QB],
                                     ctxB[0:HD, :], rbcB[:, :])
                # stage into the AllGather input buffer (gpsimd ring keeps
                # this dependent DMA off the Sync queue's in-order stream)
                c = (b * S + qt * QB) // AGCH
                nc.gpsimd.dma_start(cc_in[c][m * 128:(m + 1) * 128, :],
                                    ctx_sb[m][:, qoff:qoff + QB])
            for _ in range(budget[1]):
                if fillers:
                    fillers.popleft()()

        def ag_chunk(c):
            nc.gpsimd.collective_compute(
                "AllGather", mybir.AluOpType.bypass,
                replica_groups=[list(range(NCORES))],
                ins=[cc_in[c][:, :]],
                outs=[cc_out[c][:, :]])

        # ================= emission schedule =================
        from collections import deque

        x0 = load_xt(0, 0)
        nc.sync.dma_start(wq_sb[:, :, :], wqT)
        emit_late_consts()
        x1 = load_xt(0, 1)
        proj_compute(0, 0, x0)
        x2 = load_xt(0, 2)
        proj_compute(0, 1, x1)
        x3 = load_xt(0, 3)
        proj_compute(0, 2, x2)
        proj_compute(0, 3, x3)
        rope_k(0)
        rope_inplace(qt_sb[0], 128, 0)
        rope_inplace(qt_sb[1], 128, 0)

        # proj(b1) as filler units popped inside attention-b0 windows
        xts1 = {0: load_xt(1, 0)}

        def kv_unit(ci):
            def f():
                if ci + 1 < NCH and ci + 1 not in xts1:
                    xts1[ci + 1] = load_xt(1, ci + 1)
                proj_kv_group(1, ci, xts1[ci])
            return f

        def q_unit(ci, m):
            def f():
                proj_q_group(1, ci, m, xts1[ci])
            return f

        units = deque()
        for ci in range(NCH):
            units.append(kv_unit(ci))
            units.append(q_unit(ci, 0))
            units.append(q_unit(ci, 1))

        attn_qt(0, 0, units, (1, 1))
        ag_chunk(0)
        attn_qt(0, 1, units, (1, 2))
        ag_chunk(1)
        attn_qt(0, 2, units, (2, 2))
        ag_chunk(2)
        attn_qt(0, 3, units, (2, 1))
        ag_chunk(3)
        while units:
            units.popleft()()
        rope_k(1)
        rope_inplace(qt_sb[0], 128, 1)
        rope_inplace(qt_sb[1], 128, 1)
        ph_proj.close()

        # phase 5 pools (PSUM banks reuse the closed proj pool's)
        with ExitStack() as ph5:
            o_ps_pool = ph5.enter_context(
                tc.tile_pool(name="o_ps", bufs=2, space="PSUM"))
            ct_pool = ph5.enter_context(tc.tile_pool(name="ct", bufs=2))
            ob_pool = ph5.enter_context(tc.tile_pool(name="ob", bufs=2))
            cts = {}

            def ph5_mgroup(c, m, ct):
                tok0 = c * AGCH
                o_ps = o_ps_pool.tile([128, AGCH], f32, name="ops", tag="ops")
                for t in range(ET):
                    nc.tensor.matmul(
                        o_ps[:, :], wo_sb[:, t, m * 128:(m + 1) * 128],
                        ct[:, t, :], start=(t == 0), stop=(t == ET - 1))
                ob = ob_pool.tile([128, AGCH], f32, name="ob", tag="ob")
                nc.vector.tensor_copy(ob[:, :], o_ps[:, :])
                nc.sync.dma_start(
                    outT[m * 128:(m + 1) * 128, tok0:tok0 + AGCH], ob[:, :])

            def ct_m0_unit(c):
                def f():
                    ct = ct_pool.tile([128, ET, AGCH], bf16, name="ct",
                                      tag="ct")
                    nc.sync.dma_start(
                        ct[:, :, :],
                        cc_out[c][:, :].rearrange("(t p) i -> p t i", p=128))
                    cts[c] = ct
                    ph5_mgroup(c, 0, ct)
                return f

            def m1_unit(c):
                def f():
                    ph5_mgroup(c, 1, cts[c])
                return f

            u5 = deque()
            for c in range(NAG):
                u5.append(ct_m0_unit(c))
                u5.append(m1_unit(c))

            # fill the rope-b1 PE window with the first three Wo chunks
            for _ in range(6):
                u5.popleft()()
            # attention b1; Wo chunks popped into each window's ACT surplus.
            # A chunk's units must never be emitted before its ag_chunk: a
            # reader emitted before its writer gets no RAW dependency and
            # reads uninitialized DRAM.
            attn_qt(1, 0, u5, (1, 1))
            ag_chunk(4)
            attn_qt(1, 1, u5, (1, 1))
            ag_chunk(5)
            attn_qt(1, 2, u5, (2, 0))
            ag_chunk(6)
            attn_qt(1, 3, u5, (2, 0))
            ag_chunk(7)
            while u5:
                u5.popleft()()

            if dbg:
                for m in range(2):
                    nc.sync.dma_start(qtd[m, :, :], qt_sb[m][:, :])
                    nc.sync.dma_start(cxd[m, :, :], ctx_sb[m][:, :])
                nc.sync.dma_start(ktd[:, :], kt_sb[:, :])
                nc.sync.dma_start(vaugd[:, :, :], vaug[:, :, 0:HD + 1])

    nc.compile()
    return nc


def make_in_maps(cfg, x, cos, sin, Wq, Wk, Wv, Wo):
    """Host-side prep: transpose/slice full inputs into per-core input maps.

    All matmul operands are pre-rearranged into the on-chip [128, t, free]
    layouts so every device DMA is contiguous per partition.
    """
    import ml_dtypes
    B, S, E = cfg["B"], cfg["S"], cfg["E"]
    NH, NKV, HD, NCORES = cfg["NH"], cfg["NKV"], cfg["HD"], cfg["ncores"]
    IC = cfg["IC"]
    HPC = NH // NCORES
    QH = HPC * HD
    KVPC = NKV // NCORES
    ET = E // 128
    NCHB = (B * S) // IC

    mmnp = ml_dtypes.bfloat16
    x = np.asarray(x, dtype=np.float32)
    cos = np.asarray(cos, dtype=np.float32)
    sin = np.asarray(sin, dtype=np.float32)
    Wq = np.asarray(Wq, dtype=np.float32)
    Wk = np.asarray(Wk, dtype=np.float32)
    Wv = np.asarray(Wv, dtype=np.float32)
    Wo = np.asarray(Wo, dtype=np.float32)

    xT = x.reshape(B * S, E).T.astype(mmnp)              # [E, NI]
    xTc = np.ascontiguousarray(
        xT.reshape(ET, 128, NCHB, IC).transpose(2, 1, 0, 3))

    def wprep(w):  # [E, out] -> [128, ET, out] contiguous
        return np.ascontiguousarray(
            w.reshape(ET, 128, -1).transpose(1, 0, 2).astype(mmnp))

    cos_t = cos.T[:HD]                        # [64, S]
    cosT = np.ascontiguousarray(
        np.concatenate([cos_t, cos_t], axis=0).astype(mmnp))
    sin_t = sin.T[:HD].copy()
    sin_t[:HD // 2] *= -1.0                   # signed sin for rotate-half
    sinT = np.ascontiguousarray(
        np.concatenate([sin_t, sin_t], axis=0).astype(mmnp))

    in_maps = []
    for c in range(NCORES):
        qsl = slice(c * QH, (c + 1) * QH)
        ksl = slice(c * KVPC * HD, (c + 1) * KVPC * HD)
        wq = wprep(Wq[qsl, :].T)
        wkv = wprep(np.concatenate([Wk[ksl, :].T, Wv[ksl, :].T], axis=1))
        wo = wprep(Wo[qsl, :].T)
        in_maps.append(dict(xTc=xTc, wqT=wq, wkvT=wkv, woT=wo,
                            cosT=cosT, sinT=sinT))
    return in_maps


def assemble_output(cfg, results):
    B, S, E = cfg["B"], cfg["S"], cfg["E"]
    outT = np.concatenate([r["outT"] for r in results], axis=0)  # [E, B*S]
    return np.ascontiguousarray(outT.T.reshape(B, S, E).astype(np.float32))


def kernel(x, mask, cos, sin, Wq, Wk, Wv, Wo):
    global LAST_RESULTS
    _ensure_concourse()
    from concourse import bass_utils

    cfg = FULL_CFG
    nc = build_gqa(cfg)
    in_maps = make_in_maps(cfg, x, cos, sin, Wq, Wk, Wv, Wo)
    res = bass_utils.run_bass_kernel_spmd(
        nc, in_maps, core_ids=list(range(cfg["ncores"])))
    LAST_RESULTS = res
    return assemble_output(cfg, res.results)
